# revision 24
# baseline (speedup 1.0000x reference)
"""Trainium2 Bass kernel for CausalGraphDiscovery.

Accepts FULL (unsharded) inputs, shards data over 8 NeuronCores
(data-parallel over N = batch*seq samples), runs one SPMD Bass/Tile
kernel, gathers per-core outputs on the host.

Self-contained: all shapes hardcoded.
"""
import numpy as np

import concourse.bacc as bacc
import concourse.bass as bass
import concourse.mybir as mybir
from concourse.tile import TileContext
from concourse import bass_utils

F32 = mybir.dt.float32
F32R = mybir.dt.float32r
BF16 = mybir.dt.bfloat16
AF = mybir.ActivationFunctionType
ALU = mybir.AluOpType

NV = 16            # n_vars
BATCH = 16
SEQ = 512
NTOT = BATCH * SEQ     # 8192 samples
NCORES = 8
NS = NTOT // NCORES    # 1024 samples per core
NP = NV * (NV - 1)     # 240 ordered pairs
NG = NP // 4           # 60 groups of 4 pairs
NU = NG // 4           # 15 supergroups of 4 groups

PAIRS = [(i, j) for i in range(NV) for j in range(NV) if i != j]
PI = np.array([p[0] for p in PAIRS])
PJ = np.array([p[1] for p in PAIRS])


def _build_nc():
    nc = bacc.Bacc("TRN2", debug=False)

    def inp(name, shape):
        return nc.dram_tensor(name, shape, F32, kind="ExternalInput").ap()

    xT = inp("xT", [16, NS])          # per-core shard, transposed
    dataS = inp("dataS", [512, 256])  # data[b,s,v] -> [s, v*16+b]
    ws1d = inp("Ws1r", [16, 4096])
    ws2d = inp("Ws2r", [128, 256])
    ws3d = inp("Ws3", [128, 256])
    bs1d = inp("bs1", [1, 256])
    bs2d = inp("bs2", [1, 128])
    bs3d = inp("bs3", [1, 256])
    eyed = inp("eye128", [128, 128])
    mask16d = inp("mask16", [16, 16])
    triud = inp("triu", [1, 256])
    wm1d = inp("Wm1full", [128, 512])
    wm2d = inp("Wm2L2", [128, 2048])
    wm3d = inp("Wm3L3", [128, 128])
    bm1d = inp("bm1T", [128, 16])
    bm2d = inp("bm2p", [128, 8])
    bm3d = inp("BM3", [16, 1])
    w1bd = inp("W1B", [128, 1920])
    w2bd = inp("W2S", [128, 3840])
    bt1d = inp("bt1B", [128, 60])
    bt2d = inp("bt2q", [64, 4])

    ADJ = nc.dram_tensor("ADJ", [16, 16], F32, kind="ExternalOutput").ap()
    PRED = nc.dram_tensor("PRED", [16, NS], F32, kind="ExternalOutput").ap()
    SCACC = nc.dram_tensor("SCACC", [64, 4], F32, kind="ExternalOutput").ap()

    with TileContext(nc) as tc:
        with tc.tile_pool(name="wts", bufs=1) as wts, \
             tc.tile_pool(name="work", bufs=2) as work, \
             tc.tile_pool(name="dram", bufs=1, space="DRAM") as dpool, \
             tc.tile_pool(name="psum", bufs=2, space="PSUM") as pp:

            # ---------------- loads (pairs-critical tensors first) ----
            def load(name, src, shape=None):
                t = wts.tile(shape or list(src.shape), F32, name=name)
                nc.sync.dma_start(out=t[:], in_=src)
                return t

            def to_f32r(name, src, shape):
                t = wts.tile(shape, F32R, name=name)
                nc.gpsimd.tensor_copy(t[:], src[:])
                return t

            xT4f = wts.tile([128, NS], F32, name="xT4f")
            nc.sync.dma_start(
                out=xT4f[:], in_=xT.unsqueeze(0).broadcast_to([8, 16, NS]))
            # xT conversion on DVE (idle at startup); W1B DMA+convert in
            # halves so gpsimd conversion overlaps the second DMA half.
            xT4r = wts.tile([128, NS], F32R, name="xT4r")
            nc.vector.tensor_copy(xT4r[:], xT4f[:])
            w1bf = wts.tile([128, 1920], F32, name="w1bf")
            w1br = wts.tile([128, 1920], F32R, name="w1br")
            for hh in range(2):
                sl = slice(960 * hh, 960 * (hh + 1))
                nc.sync.dma_start(out=w1bf[:, sl], in_=w1bd[:, sl])
                nc.gpsimd.tensor_copy(w1br[:, sl], w1bf[:, sl])
            bt1B = load("bt1Bs", bt1d)
            w2bf = wts.tile([128, 3840], F32, name="w2bf")
            w2br = wts.tile([128, 3840], F32R, name="w2br")
            for hh in range(2):
                sl = slice(1920 * hh, 1920 * (hh + 1))
                nc.sync.dma_start(out=w2bf[:, sl], in_=w2bd[:, sl])
                nc.gpsimd.tensor_copy(w2br[:, sl], w2bf[:, sl])
            bt2B = load("bt2Bs", bt2d)

            dss = []
            for c in range(4):
                t = wts.tile([128, 256], F32, name=f"dataS{c}")
                nc.sync.dma_start(out=t[:], in_=dataS[128 * c:128 * (c + 1), :])
                dss.append(t)
            eye = load("eye", eyed)
            ws1 = load("ws1", ws1d)
            ws2 = load("ws2", ws2d)
            ws3 = load("ws3", ws3d)
            bs1 = load("bs1s", bs1d)
            bs2 = load("bs2s", bs2d)
            bs3 = load("bs3s", bs3d)
            mask16 = load("mask16s", mask16d)
            triu = load("trius", triud)
            wm1f = load("wm1f", wm1d)
            wm2f = load("wm2f", wm2d)
            wm3f = load("wm3f", wm3d)
            bm1T = load("bm1Ts", bm1d)
            bm2p = load("bm2ps", bm2d)
            bm3sb = load("bm3sb", bm3d)
            wm2r = to_f32r("wm2r", wm2f, [128, 2048])
            wm3r = to_f32r("wm3r", wm3f, [128, 128])

            ones128 = wts.tile([128, 1], F32, name="ones128")
            nc.vector.memset(ones128[:], 1.0)

            # ---------------- stage A: corr + structure MLP (fp32) -------
            sumS = []
            for c in range(4):
                t = wts.tile([128, 16], F32, name=f"sumS{c}")
                nc.vector.tensor_reduce(
                    t[:], dss[c][:].rearrange("p (v b) -> p v b", b=16),
                    axis=mybir.AxisListType.X, op=ALU.add)
                sumS.append(t)

            pcov = pp.tile([16, 16], F32, name="pcov", tag="m16")
            for c in range(4):
                nc.tensor.matmul(pcov[:], sumS[c][:], sumS[c][:],
                                 start=(c == 0), stop=(c == 3))
            pcs = pp.tile([1, 16], F32, name="pcs", tag="m16")
            for c in range(4):
                nc.tensor.matmul(pcs[:], ones128[:], sumS[c][:],
                                 start=(c == 0), stop=(c == 3))
            csb = work.tile([1, 16], F32, name="csb", tag="small")
            nc.scalar.copy(csb[:], pcs[:])
            pout = pp.tile([16, 16], F32, name="pout", tag="m16")
            nc.tensor.matmul(pout[:], csb[:], csb[:], start=True, stop=True)
            covr = work.tile([16, 16], F32, name="covr", tag="small")
            nc.scalar.copy(covr[:], pcov[:])
            covc = wts.tile([16, 16], F32, name="covc")
            nc.vector.scalar_tensor_tensor(
                covc[:], pout[:], -1.0 / 512.0, covr[:],
                op0=ALU.mult, op1=ALU.add)

            dscr = work.tile([16, 16], F32, name="dscr", tag="small")
            diag = wts.tile([16, 1], F32, name="diag")
            nc.vector.scalar_tensor_tensor(
                dscr[:], covc[:], 1.0, eye[:16, :16],
                op0=ALU.mult, op1=ALU.mult, accum_out=diag[:])

            pdT = pp.tile([1, 16], F32, name="pdT", tag="m16")
            nc.tensor.transpose(pdT[:], diag[:], eye[:16, :16])
            dTm = work.tile([1, 16], F32, name="dTm", tag="small")
            nc.vector.tensor_scalar_max(dTm[:], pdT[:], 0.0)
            stdT = wts.tile([1, 16], F32, name="stdT")
            nc.scalar.sqrt(stdT[:], dTm[:])
            pden = pp.tile([16, 16], F32, name="pden", tag="m16")
            nc.tensor.matmul(pden[:], stdT[:], stdT[:], start=True, stop=True)
            dens = work.tile([16, 16], F32, name="dens", tag="small")
            nc.vector.tensor_scalar_max(dens[:], pden[:], 1e-30)
            rec = work.tile([16, 16], F32, name="rec", tag="small")
            nc.vector.reciprocal(rec[:], dens[:])
            corr0 = work.tile([16, 16], F32, name="corr0", tag="small")
            nc.vector.tensor_mul(corr0[:], covc[:], rec[:])
            corrA = work.tile([16, 16], F32, name="corrA", tag="small")
            nc.scalar.activation(corrA[:], corr0[:], AF.Abs)
            corr = wts.tile([16, 16], F32, name="corr")
            nc.vector.tensor_mul(corr[:], corrA[:], mask16[:])

            # structure MLP, exact fp32
            ph1 = pp.tile([1, 256], F32, name="ph1", tag="m16")
            for j in range(16):
                nc.tensor.matmul(ph1[:], corr[:, j:j + 1],
                                 ws1[:, 256 * j:256 * (j + 1)],
                                 start=(j == 0), stop=(j == 15))
            h1b = work.tile([1, 256], F32, name="h1b", tag="small")
            nc.vector.scalar_tensor_tensor(h1b[:], ph1[:], 0.0, bs1[:],
                                           op0=ALU.add, op1=ALU.add)
            h1 = wts.tile([1, 256], F32, name="h1")
            nc.vector.tensor_scalar_max(h1[:], h1b[:], 0.0)

            pt1 = pp.tile([128, 2], F32, name="pt1", tag="m16")
            nc.tensor.transpose(pt1[:, 0:1], h1[:, :128], eye[:1, :1])
            nc.tensor.transpose(pt1[:, 1:2], h1[:, 128:], eye[:1, :1])
            h1T = wts.tile([128, 2], F32, name="h1T")
            nc.scalar.copy(h1T[:], pt1[:])

            ph2 = pp.tile([1, 128], F32, name="ph2", tag="m16")
            for c in range(2):
                nc.tensor.matmul(ph2[:], h1T[:, c:c + 1],
                                 ws2[:, 128 * c:128 * (c + 1)],
                                 start=(c == 0), stop=(c == 1))
            h2b = work.tile([1, 128], F32, name="h2b", tag="small")
            nc.vector.scalar_tensor_tensor(h2b[:], ph2[:], 0.0, bs2[:],
                                           op0=ALU.add, op1=ALU.add)
            h2 = wts.tile([1, 128], F32, name="h2")
            nc.vector.tensor_scalar_max(h2[:], h2b[:], 0.0)

            pt2 = pp.tile([128, 1], F32, name="pt2", tag="m16")
            nc.tensor.transpose(pt2[:], h2[:], eye[:1, :1])
            h2T = wts.tile([128, 1], F32, name="h2T")
            nc.scalar.copy(h2T[:], pt2[:])

            padj = pp.tile([1, 256], F32, name="padj", tag="m16")
            nc.tensor.matmul(padj[:], h2T[:], ws3[:], start=True, stop=True)
            adjb = work.tile([1, 256], F32, name="adjb", tag="small")
            nc.vector.scalar_tensor_tensor(adjb[:], padj[:], 0.0, bs3[:],
                                           op0=ALU.add, op1=ALU.add)
            adjsig = work.tile([1, 256], F32, name="adjsig", tag="small")
            nc.scalar.activation(adjsig[:], adjb[:], AF.Sigmoid)
            adjm = wts.tile([1, 256], F32, name="adjm")
            nc.vector.tensor_mul(adjm[:], adjsig[:], triu[:])

            nc.sync.dma_start(out=ADJ, in_=adjm[:])
            adjscr = dpool.tile([1, 256], F32, name="adjscr")
            nc.sync.dma_start(out=adjscr[:], in_=adjm[:])

            adj4 = wts.tile([128, 16], F32, name="adj4")
            nc.sync.dma_start(
                out=adj4[:],
                in_=adjscr[:].rearrange("a (b c) -> (a b) c", c=16)
                .unsqueeze(0).broadcast_to([8, 16, 16]))
            mt4 = wts.tile([128, 16], F32, name="mt4")
            nc.vector.tensor_scalar(mt4[:], adj4[:], 0.5, None, op0=ALU.is_gt)

            pcs16 = pp.tile([1, 16], F32, name="pcs16", tag="m16")
            nc.tensor.matmul(pcs16[:], ones128[:16, :], mt4[:16, :],
                             start=True, stop=True)
            sel01 = wts.tile([1, 16], F32, name="sel01")
            nc.vector.tensor_scalar_min(sel01[:], pcs16[:], 1.0)
            psel = pp.tile([16, 1], F32, name="psel", tag="m16")
            nc.tensor.transpose(psel[:], sel01[:], eye[:1, :1])
            selc = wts.tile([16, 1], F32, name="selc")
            nc.scalar.copy(selc[:], psel[:])

            # ---------------- pairs stage ----------------
            accA = wts.tile([64, 4], F32, name="accA")
            relu_ct = [0]

            def relu_bias(out_ap, in_ap, bias_ap):
                # alternate the PSUM->SBUF relu pass between ACT and DVE
                k = relu_ct[0]
                relu_ct[0] += 1
                if (k * 17) % 32 < 17:
                    nc.scalar.activation(out_ap, in_ap, AF.Relu, bias=bias_ap)
                else:
                    nc.vector.tensor_scalar(out_ap, in_ap, bias_ap, 0.0,
                                            op0=ALU.add, op1=ALU.max)

            # quartet a stacks 4 supergroups in psum rows 16q..16q+15
            # (a<3: u=4a+q; a=3: u=11+q, u=11 recomputed).  First and last
            # matmuls of each quartet-bank are full-width (M=64) so the psum
            # accumulation group covers all rows.
            ht_cache = {}
            for a in range(4):
                psc4 = pp.tile([64, 1024], F32, name=f"psc4_{a}",
                               tag="big2", bufs=3)
                for q in range(4):
                    u = 4 * a + q if a < 3 else 11 + q
                    hts = []
                    for w in range(4):
                        g = 4 * u + w
                        if g in ht_cache:
                            hts.append(ht_cache.pop(g))
                            continue
                        r, t = g % 4, g // 4
                        pA = pp.tile([128, 1024], F32, name=f"pA{g}",
                                     tag="big2", bufs=3)
                        for ch in range(2):
                            nc.tensor.matmul(
                                pA[:, 512 * ch:512 * (ch + 1)],
                                w1br[32 * r:32 * r + 16,
                                     128 * t:128 * (t + 1)],
                                xT4r[32 * r:32 * r + 16,
                                     512 * ch:512 * (ch + 1)],
                                start=True, stop=True,
                                tile_position=(32 * r, 0))
                        ht = work.tile([128, 1024], F32R, name=f"ht{g}",
                                       tag="ht", bufs=6)
                        relu_bias(ht[:], pA[:], bt1B[:, g:g + 1])
                        hts.append(ht)
                        if a == 2 and u == 11:
                            ht_cache[g] = ht
                    for w in range(4):
                        g = 4 * u + w
                        if q == 0 and w == 0:
                            lhs = w2br[:, 64 * g + 48:64 * g + 112]
                            m = 64
                        else:
                            lhs = w2br[:, 64 * g + 48 - 16 * q:64 * g + 64]
                            m = 16 * q + 16
                        for ch in range(2):
                            nc.tensor.matmul(
                                psc4[0:m, 512 * ch:512 * (ch + 1)],
                                lhs,
                                hts[w][:, 512 * ch:512 * (ch + 1)],
                                start=(q == 0 and w == 0),
                                stop=(q == 3 and w == 3))
                sg = work.tile([64, 1024], BF16, name=f"sg{a}", tag="sg",
                               bufs=2)
                nc.scalar.activation(sg[:], psc4[:], AF.Sigmoid,
                                     bias=bt2B[:, a:a + 1],
                                     accum_out=accA[:, a:a + 1])
            nc.sync.dma_start(out=SCACC, in_=accA[:])

            # ---------------- mech stage ----------------
            wmask = wts.tile([128, 512], F32R, name="wmask")
            for v in range(16):
                r, c = v % 4, v // 4
                nc.gpsimd.tensor_scalar_mul(
                    wmask[32 * r:32 * r + 16, 128 * c:128 * (c + 1)],
                    wm1f[32 * r:32 * r + 16, 128 * c:128 * (c + 1)],
                    mt4[32 * r:32 * r + 16, v:v + 1])

            pm16 = []
            for ch in range(2):
                t = pp.tile([16, 512], F32, name=f"pm16_{ch}", tag="m16",
                            bufs=2)
                pm16.append(t)
            for u in range(8):
                ph2m = pp.tile([128, 1024], F32, name=f"ph2m{u}",
                               tag="big2", bufs=3)
                for s01 in range(2):
                    v = 2 * u + s01
                    r, c = v % 4, v // 4
                    ph1m = pp.tile([128, 1024], F32, name=f"ph1m{v}",
                                   tag="big2", bufs=3)
                    for ch in range(2):
                        nc.tensor.matmul(
                            ph1m[:, 512 * ch:512 * (ch + 1)],
                            wmask[32 * r:32 * r + 16,
                                  128 * c:128 * (c + 1)],
                            xT4r[32 * r:32 * r + 16,
                                 512 * ch:512 * (ch + 1)],
                            start=True, stop=True,
                            tile_position=(32 * r, 0))
                    h1m = work.tile([128, 1024], F32R, name=f"h1m{v}",
                                    tag="h1m", bufs=3)
                    relu_bias(h1m[:], ph1m[:], bm1T[:, v:v + 1])
                    for ch in range(2):
                        nc.tensor.matmul(
                            ph2m[:, 512 * ch:512 * (ch + 1)],
                            wm2r[:, 128 * v:128 * (v + 1)],
                            h1m[:, 512 * ch:512 * (ch + 1)],
                            start=(s01 == 0), stop=(s01 == 1))
                h2u = work.tile([128, 1024], F32R, name=f"h2u{u}",
                                tag="h2m", bufs=2)
                relu_bias(h2u[:], ph2m[:], bm2p[:, u:u + 1])
                for ch in range(2):
                    nc.tensor.matmul(
                        pm16[ch][:],
                        wm3r[:, 16 * u:16 * (u + 1)],
                        h2u[:, 512 * ch:512 * (ch + 1)],
                        start=(u == 0), stop=(u == 7))

            for ch in range(2):
                dtl = work.tile([16, 512], F32, name=f"dtl{ch}", tag="dt",
                                bufs=2)
                nc.vector.scalar_tensor_tensor(
                    dtl[:], pm16[ch][:], bm3sb[:],
                    xT4f[0:16, 512 * ch:512 * (ch + 1)],
                    op0=ALU.add, op1=ALU.subtract)
                prt = work.tile([16, 512], F32, name=f"prt{ch}", tag="pr",
                                bufs=2)
                nc.vector.scalar_tensor_tensor(
                    prt[:], dtl[:], selc[:],
                    xT4f[0:16, 512 * ch:512 * (ch + 1)],
                    op0=ALU.mult, op1=ALU.add)
                nc.sync.dma_start(
                    out=PRED[:, 512 * ch:512 * (ch + 1)], in_=prt[:])

    nc.compile()
    return nc


def _prep_inputs(data, Ws1, bs1, Ws2, bs2, Ws3, bs3,
                 Wm1, bm1, Wm2, bm2, Wm3, bm3, Wt1, bt1, Wt2, bt2):
    f = np.float32
    flat = np.ascontiguousarray(data.reshape(NTOT, NV))

    common = {}
    common["dataS"] = np.ascontiguousarray(
        data.transpose(1, 2, 0).reshape(512, 256)).astype(f)
    common["Ws1r"] = np.ascontiguousarray(Ws1.reshape(16, 4096)).astype(f)
    common["Ws2r"] = np.ascontiguousarray(
        Ws2.reshape(2, 128, 128).transpose(1, 0, 2).reshape(128, 256)).astype(f)
    common["Ws3"] = np.ascontiguousarray(Ws3).astype(f)
    common["bs1"] = bs1.reshape(1, 256).astype(f)
    common["bs2"] = bs2.reshape(1, 128).astype(f)
    common["bs3"] = bs3.reshape(1, 256).astype(f)
    common["eye128"] = np.eye(128, dtype=f)
    common["mask16"] = (1.0 - np.eye(16)).astype(f)
    common["triu"] = np.triu(np.ones((16, 16)), 1).reshape(1, 256).astype(f)

    wm1full = np.zeros((128, 512), f)
    for v in range(NV):
        r, c = v % 4, v // 4
        for j in range(NV):
            if j == v:
                continue
            pidx = j if j < v else j - 1
            wm1full[32 * r + j, 128 * c:128 * (c + 1)] = Wm1[v, pidx, :]
    common["Wm1full"] = wm1full
    wm2l2 = np.zeros((128, 2048), f)
    for v in range(NV):
        wm2l2[:, 128 * v + 64 * (v % 2):128 * v + 64 * (v % 2) + 64] = Wm2[v]
    common["Wm2L2"] = wm2l2
    wm3l3 = np.zeros((128, 128), f)
    for v in range(NV):
        s01, u = v % 2, v // 2
        wm3l3[64 * s01:64 * (s01 + 1), 16 * u + v] = Wm3[v, :]
    common["Wm3L3"] = wm3l3
    common["bm1T"] = np.ascontiguousarray(np.asarray(bm1).T).astype(f)
    common["bm2p"] = np.ascontiguousarray(
        np.asarray(bm2).reshape(8, 128).T).astype(f)
    common["BM3"] = np.asarray(bm3).reshape(16, 1).astype(f)

    w1b = np.zeros((128, 1920), f)
    w2b = np.zeros((128, 3840), f)
    bt1b = np.zeros((128, 60), f)
    for g in range(NG):
        r, t = g % 4, g // 4
        for pl in range(4):
            p = 4 * g + pl
            i, j = PI[p], PJ[p]
            w1b[32 * r + i, 128 * t + 32 * pl:128 * t + 32 * (pl + 1)] += \
                Wt1[p, 0, :]
            w1b[32 * r + j, 128 * t + 32 * pl:128 * t + 32 * (pl + 1)] += \
                Wt1[p, 1, :]
            w2b[32 * pl:32 * (pl + 1), 64 * g + 48 + 4 * (g % 4) + pl] = \
                Wt2[p, :]
        bt1b[:, g] = np.asarray(bt1)[4 * g:4 * (g + 1), :].reshape(128)
    common["W1B"] = w1b
    common["W2S"] = w2b
    common["bt1B"] = bt1b
    bt2q = np.zeros((64, 4), f)
    for a in range(4):
        for q in range(4):
            u = 4 * a + q if a < 3 else 11 + q
            bt2q[16 * q:16 * (q + 1), a] = np.asarray(bt2)[16 * u:16 * u + 16]
    common["bt2q"] = bt2q

    xts = []
    for k in range(NCORES):
        xts.append(np.ascontiguousarray(
            flat[NS * k:NS * (k + 1), :].T).astype(f))
    return common, xts


def kernel(**inputs):
    inputs = {k: np.asarray(v) for k, v in inputs.items()}
    common, xts = _prep_inputs(**inputs)
    nc = _build_nc()
    in_maps = [dict(common, xT=xts[k]) for k in range(NCORES)]
    import os
    trace = bool(int(os.environ.get("CGD_TRACE", "0")))
    res = bass_utils.run_bass_kernel_spmd(
        nc, in_maps, core_ids=list(range(NCORES)), trace=trace)
    if trace and res.exec_time_ns is not None:
        print(f"HW exec time: {res.exec_time_ns} ns")
        if res.instructions_and_trace:
            print("trace:", res.instructions_and_trace[1])
    rs = res.results

    adj = rs[0]["ADJ"].astype(np.float32)

    preds = np.concatenate([r["PRED"].T for r in rs], axis=0)
    predictions = preds.reshape(BATCH, SEQ, NV).astype(np.float32)

    ss = np.zeros((64, 4), np.float64)
    for r in rs:
        ss += r["SCACC"].astype(np.float64)
    s = np.concatenate([ss[:, 0], ss[:, 1], ss[:, 2], ss[16:64, 3]])
    s = (s / NTOT).astype(np.float32)
    scores = np.zeros((NV, NV), np.float32)
    scores[PI, PJ] = s
    return adj, predictions, scores
